# revision 1
# baseline (speedup 1.0000x reference)
"""Trainium2 Bass kernel for nn_Conv2d_62405874811871.

Computes y[o, w] = sum_k enc_x[w, k] * weight[o, k] + bias[o], returned as
the packed vector y.reshape(-1) for enc_x [262144, 49], weight [512, 7, 7],
bias [512].

Sharding: windows are sharded across the 8 NeuronCores (32768 windows per
core); weight/bias are replicated. Each core computes all 512 output
channels for its window slice, so per-core output is a contiguous column
block of the [512, 262144] output matrix and no collectives are needed.

Per-core dataflow:
  - enc_x slice is DMA-loaded in a per-partition-contiguous layout
    (L[p, u*49+k] = xs[B*8192 + p*64 + u, k]), PE-transposed tile by tile
    into PSUM, and copied into a persistent SBUF rhs [50, 8192] whose row 49
    is constant 1.0 (so bias lands via a bias row in the stationary operand).
  - matmul: out[128ch, 512win] = wbT[50, 128].T @ rhs[50, 512] in fp32.
  - The mandatory PSUM->SBUF copies (DMA cannot read PSUM) alternate
    between VectorE and ScalarE and simultaneously un-shuffle the window
    permutation introduced by the load layout, so the staged tile is in
    natural window order and the output DMA is fully contiguous (4MB per
    transfer).
"""

import numpy as np

import concourse.mybir as mybir
import concourse.tile as tile
from concourse import bacc
from concourse.bass_utils import run_bass_kernel_spmd

F32 = mybir.dt.float32

W_TOTAL = 262144  # total windows
N_CORES = 8
W = W_TOTAL // N_CORES  # 32768 windows per core
K = 49  # kh*kw contraction
KB = K + 1  # + ones/bias row
O = 512  # out channels
G = O // 128  # channel groups of 128 partitions
# superblock sizes (windows): small first blocks so the first output DMAs
# issue within a few microseconds instead of waiting for a full 8192-window
# transpose phase; 4MB DMAs once the pipeline is saturated
SBS = [1024, 1024, 2048, 8192, 8192, 8192, 4096]
assert sum(SBS) == W


def _build(
    sbs=None,
    load_bufs=3,
    rhs_bufs=3,
    stage_bufs=2,
    tp_bufs=2,
    mm_bufs=3,
    loop_n=1,
    mode="full",
    static_rhs=True,
):
    """loop_n > 1 repeats the whole dataflow in an on-device loop (same
    output every iteration) — used only for steady-state benchmarking.
    mode: "full" | "no_out" (skip output DMAs) | "dma_only" (only output
    DMAs from a constant staging tile) — benchmarking modes."""
    sbs = SBS if sbs is None else sbs
    assert sum(sbs) == W
    nc = bacc.Bacc("TRN2", target_bir_lowering=False, debug=False, num_devices=N_CORES)
    xs = nc.dram_tensor("xs", [W, K], F32, kind="ExternalInput").ap()
    wb = nc.dram_tensor("wb", [KB, O], F32, kind="ExternalInput").ap()
    ident = nc.dram_tensor("ident", [128, 128], F32, kind="ExternalInput").ap()
    out = nc.dram_tensor("out", [O, W], F32, kind="ExternalOutput").ap()

    with tile.TileContext(nc) as tc:
        with (
            tc.tile_pool(name="const", bufs=1) as const_pool,
            tc.tile_pool(name="load", bufs=load_bufs) as load_pool,
            tc.tile_pool(name="rhs", bufs=rhs_bufs) as rhs_pool,
            tc.tile_pool(name="stage", bufs=stage_bufs) as stage_pool,
            tc.tile_pool(name="tpp", bufs=tp_bufs, space="PSUM") as tp_psum,
            tc.tile_pool(name="mmp", bufs=mm_bufs, space="PSUM") as mm_psum,
        ):
            # weights + bias row replicated at partitions 0-49 and 64-113 so
            # two matmuls can run concurrently on disjoint PE row groups
            wb_t = const_pool.tile([64 + KB, O], F32)
            nc.gpsimd.dma_start(out=wb_t[0:KB, :], in_=wb[:])
            nc.gpsimd.dma_start(out=wb_t[64 : 64 + KB, :], in_=wb[:])
            id_t = const_pool.tile([128, 128], F32)
            nc.gpsimd.dma_start(out=id_t[:], in_=ident[:])

            if mode == "dma_only":
                S0 = const_pool.tile([128, max(sbs)], F32)
                nc.vector.memset(S0[:], 1.0)

            rhs_static = None
            if static_rhs:
                # static rhs buffers: the ones-rows 49/113 (bias rows) are
                # written once here; the per-superblock copies only ever
                # touch rows 0-48 / 64-112, so no per-iteration memsets.
                rhs_static = [
                    const_pool.tile([64 + KB, max(sbs)], F32, name=f"rhsS{i}", tag=f"rhs{i}")
                    for i in range(rhs_bufs)
                ]
                for r in rhs_static:
                    nc.gpsimd.memset(r[32:KB, :], 1.0)
                    nc.gpsimd.memset(r[96 : 64 + KB, :], 1.0)

            def dma_body():
                w0 = 0
                for B, sb in enumerate(sbs):
                    for g in range(G):
                        nc.sync.dma_start(
                            out=out[g * 128 : (g + 1) * 128, w0 : w0 + sb],
                            in_=S0[:, :sb],
                        )
                    w0 += sb

            def body():
                copy_idx = 0
                w0 = 0  # window offset of the current superblock
                for B, sb in enumerate(sbs):
                    ub = sb // 128  # rows per partition in this superblock
                    # xs rows in per-partition-contiguous order:
                    #   L[p, u*K + k] = xs[w0 + p*ub + u, k]
                    xb = xs[w0 : w0 + sb, :].rearrange(
                        "(p u) k -> p (u k)", p=128, u=ub
                    )
                    L = load_pool.tile([128, ub * K], F32)
                    nc.gpsimd.dma_start(out=L[:], in_=xb)
                    if mode == "load_only":
                        w0 += sb
                        continue

                    # rhs col c = u*128 + p  <->  window w0 + p*ub + u
                    # even 512-col chunks live at partitions 0-49, odd chunks at
                    # 64-113, feeding two concurrent PE row groups.
                    if static_rhs:
                        rhs_t = rhs_static[B % rhs_bufs][:, :sb]
                    else:
                        rhs_t = rhs_pool.tile([64 + KB, sb], F32)
                        # engines need 32-aligned start partitions: memset
                        # rows 32-49 / 96-113 to 1.0; rows 32-48 / 96-112 are
                        # then overwritten by the transpose copies, leaving
                        # only the ones-rows 49 and 113 (bias rows) at 1.0.
                        nc.gpsimd.memset(rhs_t[32:KB, :], 1.0)
                        nc.gpsimd.memset(rhs_t[96 : 64 + KB, :], 1.0)
                    for q in range(ub // 4):
                        tp = tp_psum.tile([K, 512], F32)
                        for j in range(4):
                            u = q * 4 + j
                            nc.tensor.transpose(
                                tp[:, j * 128 : (j + 1) * 128],
                                L[:, u * K : (u + 1) * K],
                                id_t[:],
                            )
                        cols = slice(q * 512, (q + 1) * 512)
                        dst = rhs_t[0:K, cols]
                        if copy_idx % 2 == 0:
                            nc.vector.tensor_copy(dst, tp[:, :])
                        else:
                            nc.scalar.copy(dst, tp[:, :])
                        copy_idx += 1
                    if mode != "rhs_norepl":
                        # odd chunks feed the high PE row group: replicate to
                        # partitions 64-112 in one batched strided DMA
                        # (PSUM transpose outputs must land at partition 0,
                        # but SBUF->SBUF DMA can shift partitions)
                        nq = ub // 8  # number of odd 512-col chunks
                        src_v = rhs_t[0:K, 512 : sb].rearrange(
                            "a (qq c) -> a qq c", qq=2 * nq - 1, c=512
                        )[:, ::2, :]
                        dst_v = rhs_t[64 : 64 + K, 512 : sb].rearrange(
                            "a (qq c) -> a qq c", qq=2 * nq - 1, c=512
                        )[:, ::2, :]
                        nc.scalar.dma_start(out=dst_v, in_=src_v)

                    if mode in ("rhs_only", "rhs_norepl"):
                        w0 += sb
                        continue
                    for g in range(G):
                        S = stage_pool.tile([128, sb], F32)
                        # natural window order view: free index w' = p*ub + uu
                        Sv = S.rearrange("a (p uu) -> a uu p", p=128, uu=ub)
                        for m in range(sb // 1024):
                            P = mm_psum.tile([128, 1024], F32)
                            for h in range(2):
                                c0 = m * 1024 + h * 512
                                lo = 0 if h == 0 else 64
                                nc.tensor.matmul(
                                    P[:, h * 512 : (h + 1) * 512],
                                    wb_t[lo : lo + KB, g * 128 : (g + 1) * 128],
                                    rhs_t[lo : lo + KB, c0 : c0 + 512],
                                    start=True,
                                    stop=True,
                                    tile_position=(lo, 0),
                                )
                            if mode == "no_copy":
                                copy_idx += 1
                                continue
                            # un-shuffle: P col du*128+p -> S col p*ub + (m*8+du)
                            if mode == "simple_copy":
                                Pv = P[:, :]
                                dst = S[:, m * 1024 : (m + 1) * 1024]
                            else:
                                Pv = P.rearrange("a (du p) -> a du p", du=8, p=128)
                                dst = Sv[:, m * 8 : (m + 1) * 8, :]
                            if copy_idx % 2 == 0:
                                nc.vector.tensor_copy(dst, Pv)
                            else:
                                nc.scalar.copy(dst, Pv)
                            copy_idx += 1
                        if mode not in ("no_out", "no_copy"):
                            nc.sync.dma_start(
                                out=out[g * 128 : (g + 1) * 128, w0 : w0 + sb],
                                in_=S[:],
                            )
                    w0 += sb

            use_body = dma_body if mode == "dma_only" else body
            if loop_n == 1:
                use_body()
            else:
                with tc.For_i(0, loop_n, 1):
                    use_body()
    nc.compile()
    return nc


_NC = None


def _get_nc():
    global _NC
    if _NC is None:
        _NC = _build()
    return _NC


def _prep_inputs(enc_x, weight, bias):
    enc_x = np.ascontiguousarray(np.asarray(enc_x, dtype=np.float32))
    w_flat = np.asarray(weight, dtype=np.float32).reshape(O, -1)  # [512, 49]
    b = np.asarray(bias, dtype=np.float32)
    wb = np.concatenate([w_flat.T, b[None, :]], axis=0)  # [50, 512]
    wb = np.ascontiguousarray(wb)
    ident = np.eye(128, dtype=np.float32)
    in_maps = [
        {
            "xs": np.ascontiguousarray(enc_x[c * W : (c + 1) * W]),
            "wb": wb,
            "ident": ident,
        }
        for c in range(N_CORES)
    ]
    return in_maps


def kernel(enc_x, weight, bias, windows_nb):
    assert int(windows_nb) == W_TOTAL
    nc = _get_nc()
    in_maps = _prep_inputs(enc_x, weight, bias)
    res = run_bass_kernel_spmd(nc, in_maps, core_ids=list(range(N_CORES)))
    parts = [res.results[c]["out"] for c in range(N_CORES)]  # each [512, 32768]
    full = np.concatenate(parts, axis=1)  # [512, 262144]
    return np.ascontiguousarray(full.reshape(-1))

